# revision 12
# baseline (speedup 1.0000x reference)
"""Trainium2 Bass kernel for nn_CustomPartiallyConnectedLayer (segment_reduce).

out[b, j] = sum_c x[b, j*128 + c] * w[j*128 + c] + bias[j]
x: [2048, 65536] f32, w: [65536] f32, bias: [512] f32 -> out: [2048, 512] f32

Sharding: batch across 8 cores (256 rows each). Within a core the 256 rows
are split across two compute paths that run concurrently under the DMA
stream (the kernel is memory-bound: 64 MiB of x per core):

- DVE path (rows 0..127 of the shard, natural layout): fused
  tensor_tensor_reduce (multiply + segment-sum + bias init) per
  [128 groups x 128 c] tile.
- PE path (rows 128..255, host-transposed layout [c, j, b]): per group j a
  matmul with stationary xT_j [128c x 128b] and moving w_j [128c x 1]
  accumulating psum[:, j] -> psum [128 b x 512 j] in natural output
  layout; one DVE bias-add; contiguous store.
"""
import os
import sys
from contextlib import ExitStack

import numpy as np

sys.path.insert(0, os.path.dirname(os.path.abspath(__file__)))

import concourse.bass as bass  # noqa: E402
import concourse.tile as tile  # noqa: E402
from concourse import mybir  # noqa: E402
from concourse.bass_utils import run_bass_kernel_spmd  # noqa: E402

# --- walrus compat: split multi-wait tail drains (see tile_compat.py) ---
from concourse.vector_clock import ScopedClock  # noqa: E402


def _patched_drain_and_barrier(self, tick_clock, wait_clock):
    nc = self.nc
    drain_inst = nc.sync.drain()
    wait_clock.add_sem_waits(
        drain_inst.ins, ScopedClock({None: tick_clock.global_clock})
    )
    si = drain_inst.ins.sync_info
    if si is not None and si.on_wait is not None and len(si.on_wait) > 1:
        extra = list(si.on_wait[1:])
        del si.on_wait[1:]
        for w in extra:
            d2 = nc.sync.drain()
            d2.ins.sync_info = mybir.SyncInfo(on_wait=[w], on_update=[])

    nc.all_engine_barrier()
    assert self.sems is not None
    popped = nc._tile_sem_poison_stack.pop()
    assert popped is self._sem_poison
    nc.clear_and_free_semaphores(list(self.sems.allocated().values()))
    nc.all_engine_barrier()


tile.TileContext._drain_and_barrier = _patched_drain_and_barrier


def _split_multi_waits(nc, max_waits=1):
    """This walrus build allows at most one sem-wait per instruction.

    Tile's scheduler attaches several. Move the excess onto injected
    single-wait NoOps immediately before the instruction (same engine,
    same stream position => identical semantics).
    """
    ctr = 0
    for fn in nc.m.functions:
        for blk in fn.blocks:
            newl = []
            for inst in blk.instructions:
                si = inst.sync_info
                if (
                    si is not None
                    and si.on_wait is not None
                    and len(si.on_wait) > max_waits
                ):
                    waits = list(si.on_wait)
                    keep = waits[-max_waits:]
                    extra = waits[:-max_waits]
                    del si.on_wait[:]
                    si.on_wait.extend(keep)
                    for k in range(0, len(extra), max_waits):
                        nop = mybir.InstNoOp(
                            name=f"waitsplit_{ctr}", ins=[], outs=[]
                        )
                        ctr += 1
                        nop.engine = inst.engine
                        nop.sync_info = mybir.SyncInfo(
                            on_wait=extra[k:k + max_waits], on_update=[]
                        )
                        newl.append(nop)
                newl.append(inst)
            blk.instructions = newl
# -----------------------------------------------------------------------

N_CORES = 8
B, H1, H2, CS = 2048, 65536, 512, 128
BC = B // N_CORES          # 256 rows per core
BD = 128                   # rows on the DVE path
BP = BC - BD               # rows on the PE path (128)
NCH_D = 16                 # DVE chunks: [128p, 4096] each = 8 rows
FD = H1 // 16              # 4096 free elems per partition per chunk
JS = FD // CS              # 32 segments per partition per chunk
NCH_P = 16                 # PE chunks: 32 groups x 128 b each
JCH = H2 // NCH_P          # 32 groups per PE chunk

F32 = mybir.dt.float32
BF16 = mybir.dt.bfloat16


def _build_bass(repeat: int = 1):
    """repeat>1 replays the whole streaming body that many times inside the
    module — timing-only builds so async-rate slope over repeat cancels
    per-dispatch overhead. kernel() always uses repeat=1."""
    nc = bass.Bass(trn_type="TRN2", target_bir_lowering=False)

    x_nat = nc.dram_tensor("x_nat", [NCH_D, 128, FD], BF16, kind="ExternalInput").ap()
    x_t = nc.dram_tensor("x_t", [128, H2 * BP], BF16, kind="ExternalInput").ap()
    w_rep = nc.dram_tensor("w_rep", [128, FD], BF16, kind="ExternalInput").ap()
    w_t = nc.dram_tensor("w_t", [128, H2], BF16, kind="ExternalInput").ap()
    bias_rep = nc.dram_tensor("bias_rep", [128, JS], F32, kind="ExternalInput").ap()
    bias_b = nc.dram_tensor("bias_b", [128, H2], F32, kind="ExternalInput").ap()
    out_d = nc.dram_tensor("out_d", [NCH_D, 128, JS], F32, kind="ExternalOutput").ap()
    out_p = nc.dram_tensor("out_p", [BP, H2], F32, kind="ExternalOutput").ap()

    with tile.TileContext(nc) as tc, ExitStack() as ctx:
        consts = ctx.enter_context(tc.tile_pool(name="consts", bufs=1))
        xn_pool = ctx.enter_context(tc.tile_pool(name="xn", bufs=3))
        xt_pool = ctx.enter_context(tc.tile_pool(name="xt", bufs=3))
        scratch_pool = ctx.enter_context(tc.tile_pool(name="scratch", bufs=2))
        res_pool = ctx.enter_context(tc.tile_pool(name="res", bufs=2))
        out_pool = ctx.enter_context(tc.tile_pool(name="outp", bufs=1))
        psum_pool = ctx.enter_context(tc.tile_pool(name="psum", bufs=1, space="PSUM"))

        w_rep_sb = consts.tile([128, FD], BF16)
        nc.gpsimd.dma_start(w_rep_sb[:], w_rep[:])
        w_t_sb = consts.tile([128, H2], BF16)
        nc.gpsimd.dma_start(w_t_sb[:], w_t[:])
        bias_rep_sb = consts.tile([128, JS], F32)
        nc.gpsimd.dma_start(bias_rep_sb[:], bias_rep[:])
        bias_b_sb = consts.tile([128, H2], F32)
        nc.gpsimd.dma_start(bias_b_sb[:], bias_b[:])

        psum_t = psum_pool.tile([128, H2], F32)

        for _rep in range(repeat):
          for ci in range(max(NCH_D, NCH_P)):
            if ci < NCH_D:
                # ---- DVE path chunk: 8 batch rows, fused mul+segsum ----
                xn = xn_pool.tile([128, FD], BF16)
                nc.sync.dma_start(xn[:], x_nat[ci])
                prod = scratch_pool.tile([128, FD], BF16)
                nc.vector.tensor_mul(prod[:], xn[:], w_rep_sb[:])
                res_raw = res_pool.tile([128, JS], F32)
                nc.vector.tensor_reduce(
                    res_raw[:],
                    prod[:].rearrange("p (j c) -> p j c", c=CS),
                    axis=mybir.AxisListType.X,
                    op=mybir.AluOpType.add,
                )
                res = res_pool.tile([128, JS], F32)
                nc.vector.tensor_add(res[:], res_raw[:], bias_rep_sb[:])
                nc.gpsimd.dma_start(out_d[ci], res[:])

            if ci < NCH_P:
                # ---- PE path chunk: 32 groups x all 128 PE-rows ----
                xt = xt_pool.tile([128, JCH * BP], BF16)
                j0 = ci * JCH
                nc.scalar.dma_start(xt[:], x_t[:, j0 * BP:(j0 + JCH) * BP])
                for jj in range(JCH):
                    j = j0 + jj
                    nc.tensor.matmul(
                        out=psum_t[:, j:j + 1],
                        lhsT=xt[:, jj * BP:(jj + 1) * BP],
                        rhs=w_t_sb[:, j:j + 1],
                        start=True,
                        stop=True,
                    )

        out_sb = out_pool.tile([128, H2], F32)
        nc.vector.tensor_add(out_sb[:], psum_t[:], bias_b_sb[:])
        nc.gpsimd.dma_start(out_p[:], out_sb[:])

    _split_multi_waits(nc)
    return nc


_CACHE = {}


def _get_nc(repeat: int = 1):
    if repeat not in _CACHE:
        _CACHE[repeat] = _build_bass(repeat)
    return _CACHE[repeat]


def make_in_maps(x, weights, bias):
    """Host-side shard + relayout + bf16 cast. Returns per-core input dicts."""
    import ml_dtypes

    bf16 = ml_dtypes.bfloat16
    x = np.asarray(x, dtype=np.float32).astype(bf16)
    w = np.asarray(weights, dtype=np.float32).astype(bf16)
    b = np.ascontiguousarray(bias, dtype=np.float32)

    w_rep = np.tile(w.reshape(16, FD), (8, 1))            # [128, 4096] bf16
    w_t = np.ascontiguousarray(w.reshape(H2, CS).T)       # [128, 512] bf16
    bias_rep = np.tile(b.reshape(16, JS), (8, 1))         # [128, 32]
    bias_b = np.broadcast_to(b, (128, H2)).copy()         # [128, 512]

    in_maps = []
    for c in range(N_CORES):
        xs = x[c * BC:(c + 1) * BC]
        x_d = xs[:BD]
        x_p = xs[BD:]
        x_nat = np.ascontiguousarray(x_d).reshape(NCH_D, 128, FD)
        x_t = np.ascontiguousarray(
            x_p.reshape(BP, H2, CS).transpose(2, 1, 0)
        ).reshape(128, H2 * BP)
        in_maps.append({
            "x_nat": x_nat, "x_t": x_t,
            "w_rep": w_rep, "w_t": w_t,
            "bias_rep": bias_rep, "bias_b": bias_b,
        })
    return in_maps


def assemble_out(results):
    out = np.empty((B, H2), np.float32)
    for c in range(N_CORES):
        od = results[c]["out_d"].reshape(BD, H2)
        op = results[c]["out_p"]
        out[c * BC:c * BC + BD] = od
        out[c * BC + BD:(c + 1) * BC] = op
    return out


def kernel(x, weights, bias):
    nc = _get_nc()
    in_maps = make_in_maps(x, weights, bias)
    res = run_bass_kernel_spmd(nc, in_maps, list(range(N_CORES)), trace=False)
    return assemble_out(res.results)


if __name__ == "__main__":
    rng = np.random.default_rng(0)
    x = rng.standard_normal((B, H1), dtype=np.float32)
    w = rng.standard_normal(H1, dtype=np.float32)
    b = rng.standard_normal(H2, dtype=np.float32)
    got = kernel(x, weights=w, bias=b)
    want = (x * w).reshape(B, H2, CS).sum(-1) + b
    denom = np.abs(want).max()
    print("abs err:", np.abs(got - want).max(), "rel:", np.abs(got - want).max() / denom)



# revision 17
# speedup vs baseline: 1.7833x; 1.7833x over previous
"""Trainium2 Bass kernel for nn_CustomPartiallyConnectedLayer (segment_reduce).

out[b, j] = sum_c x[b, j*128 + c] * w[j*128 + c] + bias[j]
x: [2048, 65536] f32, w: [65536] f32, bias: [512] f32 -> out: [2048, 512] f32

Sharding: batch across 8 cores (256 rows each). Within a core the 256 rows
are split across two compute paths that run concurrently under the DMA
stream (the kernel is memory-bound: 64 MiB of x per core):

- DVE path (rows 0..127 of the shard, natural layout): fused
  tensor_tensor_reduce (multiply + segment-sum + bias init) per
  [128 groups x 128 c] tile.
- PE path (rows 128..255, host-transposed layout [c, j, b]): per group j a
  matmul with stationary xT_j [128c x 128b] and moving w_j [128c x 1]
  accumulating psum[:, j] -> psum [128 b x 512 j] in natural output
  layout; one DVE bias-add; contiguous store.
"""
import os
import sys
from contextlib import ExitStack

import numpy as np

sys.path.insert(0, os.path.dirname(os.path.abspath(__file__)))

import concourse.bass as bass  # noqa: E402
import concourse.tile as tile  # noqa: E402
from concourse import mybir  # noqa: E402
from concourse.bass_utils import run_bass_kernel_spmd  # noqa: E402

# --- walrus compat: split multi-wait tail drains (see tile_compat.py) ---
from concourse.vector_clock import ScopedClock  # noqa: E402


def _patched_drain_and_barrier(self, tick_clock, wait_clock):
    nc = self.nc
    drain_inst = nc.sync.drain()
    wait_clock.add_sem_waits(
        drain_inst.ins, ScopedClock({None: tick_clock.global_clock})
    )
    si = drain_inst.ins.sync_info
    if si is not None and si.on_wait is not None and len(si.on_wait) > 1:
        extra = list(si.on_wait[1:])
        del si.on_wait[1:]
        for w in extra:
            d2 = nc.sync.drain()
            d2.ins.sync_info = mybir.SyncInfo(on_wait=[w], on_update=[])

    nc.all_engine_barrier()
    assert self.sems is not None
    popped = nc._tile_sem_poison_stack.pop()
    assert popped is self._sem_poison
    nc.clear_and_free_semaphores(list(self.sems.allocated().values()))
    nc.all_engine_barrier()


tile.TileContext._drain_and_barrier = _patched_drain_and_barrier


def _split_multi_waits(nc, max_waits=1):
    """This walrus build allows at most one sem-wait per instruction.

    Tile's scheduler attaches several. Move the excess onto injected
    single-wait NoOps immediately before the instruction (same engine,
    same stream position => identical semantics).
    """
    ctr = 0
    for fn in nc.m.functions:
        for blk in fn.blocks:
            newl = []
            for inst in blk.instructions:
                si = inst.sync_info
                if (
                    si is not None
                    and si.on_wait is not None
                    and len(si.on_wait) > max_waits
                ):
                    waits = list(si.on_wait)
                    keep = waits[-max_waits:]
                    extra = waits[:-max_waits]
                    del si.on_wait[:]
                    si.on_wait.extend(keep)
                    for k in range(0, len(extra), max_waits):
                        nop = mybir.InstNoOp(
                            name=f"waitsplit_{ctr}", ins=[], outs=[]
                        )
                        ctr += 1
                        nop.engine = inst.engine
                        nop.sync_info = mybir.SyncInfo(
                            on_wait=extra[k:k + max_waits], on_update=[]
                        )
                        newl.append(nop)
                newl.append(inst)
            blk.instructions = newl
# -----------------------------------------------------------------------

N_CORES = 8
B, H1, H2, CS = 2048, 65536, 512, 128
BC = B // N_CORES          # 256 rows per core
BD = 64                    # rows on the DVE path
BP = BC - BD               # rows on the PE path (192)
NCH_D = BD // 8            # DVE chunks: [128p, 4096] each = 8 rows
FD = H1 // 16              # 4096 free elems per partition per chunk
JS = FD // CS              # 32 segments per partition per chunk
NCH_P = 16                 # PE chunks: 32 groups x all BP rows each
JCH = H2 // NCH_P          # 32 groups per PE chunk
BB0 = 128                  # PE b-block sizes (psum partition limit)
BB1 = BP - BB0             # 64

F32 = mybir.dt.float32
BF16 = mybir.dt.bfloat16


def _build_bass(repeat: int = 1):
    """repeat>1 replays the whole streaming body that many times inside the
    module — timing-only builds so async-rate slope over repeat cancels
    per-dispatch overhead. kernel() always uses repeat=1."""
    nc = bass.Bass(trn_type="TRN2", target_bir_lowering=False)

    x_nat = nc.dram_tensor("x_nat", [NCH_D, 128, FD], BF16, kind="ExternalInput").ap()
    x_t = nc.dram_tensor("x_t", [128, H2 * BP], BF16, kind="ExternalInput").ap()
    w_rep = nc.dram_tensor("w_rep", [128, FD], BF16, kind="ExternalInput").ap()
    w_t = nc.dram_tensor("w_t", [128, H2], BF16, kind="ExternalInput").ap()
    bias_rep = nc.dram_tensor("bias_rep", [128, JS], F32, kind="ExternalInput").ap()
    bias_b = nc.dram_tensor("bias_b", [128, H2], F32, kind="ExternalInput").ap()
    out_d = nc.dram_tensor("out_d", [NCH_D, 128, JS], F32, kind="ExternalOutput").ap()
    out_p = nc.dram_tensor("out_p", [BP, H2], F32, kind="ExternalOutput").ap()

    with tile.TileContext(nc) as tc, ExitStack() as ctx:
        consts = ctx.enter_context(tc.tile_pool(name="consts", bufs=1))
        xn_pool = ctx.enter_context(tc.tile_pool(name="xn", bufs=3))
        xt_pool = ctx.enter_context(tc.tile_pool(name="xt", bufs=3))
        scratch_pool = ctx.enter_context(tc.tile_pool(name="scratch", bufs=2))
        res_pool = ctx.enter_context(tc.tile_pool(name="res", bufs=2))
        out_pool = ctx.enter_context(tc.tile_pool(name="outp", bufs=1))
        psum_pool = ctx.enter_context(tc.tile_pool(name="psum", bufs=1, space="PSUM"))

        w_rep_sb = consts.tile([128, FD], BF16)
        nc.gpsimd.dma_start(w_rep_sb[:], w_rep[:])
        w_t_sb = consts.tile([128, H2], BF16)
        nc.gpsimd.dma_start(w_t_sb[:], w_t[:])
        bias_rep_sb = consts.tile([128, JS], F32)
        nc.gpsimd.dma_start(bias_rep_sb[:], bias_rep[:])
        bias_b_sb = consts.tile([128, H2], F32)
        nc.gpsimd.dma_start(bias_b_sb[:], bias_b[:])

        psum_t = psum_pool.tile([128, H2], F32)
        psum_t1 = psum_pool.tile([BB1, H2], F32)
        xt_eng = [nc.scalar, nc.sync, nc.scalar, nc.gpsimd]

        for _rep in range(repeat):
          for ci in range(max(NCH_D, NCH_P)):
            if ci < NCH_D:
                # ---- DVE path chunk: 8 batch rows, fused mul+segsum ----
                xn = xn_pool.tile([128, FD], BF16)
                nc.sync.dma_start(xn[:], x_nat[ci])
                prod = scratch_pool.tile([128, FD], BF16)
                nc.vector.tensor_mul(prod[:], xn[:], w_rep_sb[:])
                # segment sum as a binary add tree: TensorTensor runs in the
                # DVE 2x perf mode (TensorReduce never does), so halving down
                # to 4-wide costs ~half the cycles of one full reduce.
                cur, width = prod, CS
                with nc.allow_low_precision(reason="bf16 tree partials"):
                    while width > 4:
                        width //= 2
                        nxt = res_pool.tile([128, JS * width], BF16)
                        cv = cur[:].rearrange("p (j c) -> p j c", c=2 * width)
                        nc.vector.tensor_add(
                            nxt[:].rearrange("p (j c) -> p j c", c=width),
                            cv[:, :, 0:width],
                            cv[:, :, width:2 * width],
                        )
                        cur = nxt
                res_raw = res_pool.tile([128, JS], F32)
                nc.vector.tensor_reduce(
                    res_raw[:],
                    cur[:].rearrange("p (j h) -> p j h", h=width),
                    axis=mybir.AxisListType.X,
                    op=mybir.AluOpType.add,
                )
                res = res_pool.tile([128, JS], F32)
                nc.vector.tensor_add(res[:], res_raw[:], bias_rep_sb[:])
                nc.gpsimd.dma_start(out_d[ci], res[:])

            if ci < NCH_P:
                # ---- PE path chunk: 32 groups x all BP PE-rows ----
                xt = xt_pool.tile([128, JCH * BP], BF16)
                j0 = ci * JCH
                xt_eng[ci % len(xt_eng)].dma_start(
                    xt[:], x_t[:, j0 * BP:(j0 + JCH) * BP]
                )
                for jj in range(JCH):
                    j = j0 + jj
                    nc.tensor.matmul(
                        out=psum_t[:, j:j + 1],
                        lhsT=xt[:, jj * BP:jj * BP + BB0],
                        rhs=w_t_sb[:, j:j + 1],
                        start=True,
                        stop=True,
                    )
                    nc.tensor.matmul(
                        out=psum_t1[:, j:j + 1],
                        lhsT=xt[:, jj * BP + BB0:(jj + 1) * BP],
                        rhs=w_t_sb[:, j:j + 1],
                        start=True,
                        stop=True,
                    )

        out_sb = out_pool.tile([128, H2], F32)
        nc.vector.tensor_add(out_sb[:], psum_t[:], bias_b_sb[:])
        nc.gpsimd.dma_start(out_p[:BB0], out_sb[:])
        out_sb1 = out_pool.tile([BB1, H2], F32)
        nc.vector.tensor_add(out_sb1[:], psum_t1[:], bias_b_sb[:BB1])
        nc.gpsimd.dma_start(out_p[BB0:], out_sb1[:])

    _split_multi_waits(nc)
    return nc


_CACHE = {}


def _get_nc(repeat: int = 1):
    if repeat not in _CACHE:
        _CACHE[repeat] = _build_bass(repeat)
    return _CACHE[repeat]


def make_in_maps(x, weights, bias):
    """Host-side shard + relayout + bf16 cast. Returns per-core input dicts."""
    import ml_dtypes

    bf16 = ml_dtypes.bfloat16
    x = np.asarray(x, dtype=np.float32).astype(bf16)
    w = np.asarray(weights, dtype=np.float32).astype(bf16)
    b = np.ascontiguousarray(bias, dtype=np.float32)

    w_rep = np.tile(w.reshape(16, FD), (8, 1))            # [128, 4096] bf16
    w_t = np.ascontiguousarray(w.reshape(H2, CS).T)       # [128, 512] bf16
    bias_rep = np.tile(b.reshape(16, JS), (8, 1))         # [128, 32]
    bias_b = np.broadcast_to(b, (128, H2)).copy()         # [128, 512]

    in_maps = []
    for c in range(N_CORES):
        xs = x[c * BC:(c + 1) * BC]
        x_d = xs[:BD]
        x_p = xs[BD:]
        x_nat = np.ascontiguousarray(x_d).reshape(NCH_D, 128, FD)
        x_t = np.ascontiguousarray(
            x_p.reshape(BP, H2, CS).transpose(2, 1, 0)
        ).reshape(128, H2 * BP)
        in_maps.append({
            "x_nat": x_nat, "x_t": x_t,
            "w_rep": w_rep, "w_t": w_t,
            "bias_rep": bias_rep, "bias_b": bias_b,
        })
    return in_maps


def assemble_out(results):
    out = np.empty((B, H2), np.float32)
    for c in range(N_CORES):
        od = results[c]["out_d"].reshape(BD, H2)
        op = results[c]["out_p"]
        out[c * BC:c * BC + BD] = od
        out[c * BC + BD:(c + 1) * BC] = op
    return out


def kernel(x, weights, bias):
    nc = _get_nc()
    in_maps = make_in_maps(x, weights, bias)
    res = run_bass_kernel_spmd(nc, in_maps, list(range(N_CORES)), trace=False)
    return assemble_out(res.results)


if __name__ == "__main__":
    rng = np.random.default_rng(0)
    x = rng.standard_normal((B, H1), dtype=np.float32)
    w = rng.standard_normal(H1, dtype=np.float32)
    b = rng.standard_normal(H2, dtype=np.float32)
    got = kernel(x, weights=w, bias=b)
    want = (x * w).reshape(B, H2, CS).sum(-1) + b
    denom = np.abs(want).max()
    print("abs err:", np.abs(got - want).max(), "rel:", np.abs(got - want).max() / denom)

